# revision 17
# baseline (speedup 1.0000x reference)
"""Trainium2 Bass kernel for location-sensitive attention (content score +
1-channel conv location score + softmax + weighted sum), data-parallel over
the batch dim across 8 NeuronCores.

Per core (4 batches): for each batch b
  prod[t,h] = eh[t,h] * dhx[h]                  (DVE mult, f32 -> scratch)
  pax[t]    = sum_h prod[t,h] + conv(ax)[t]     (ACT Copy w/ fp32 accum_out;
                                                 the Copy's main output is the
                                                 SAME product cast to bf16)
  ax_new    = softmax(pax)                      (ACT exp+rowsum fused, PE for
                                                 cross-partition folds)
  sx[h]     = (sum_t ax_new[t] * prod_bf16[t,h]) / dhx[h]
                                                (PE bf16 matmul over the bf16
                                                 product; one divide at the end
                                                 undoes the dhx factor)

eh is streamed from HBM exactly once (8 MiB/batch); only the bf16 product stays
resident for the weighted sum. DMA of batch b+1 overlaps compute of batch b.
"""

import sys

for _p in ("/opt/trn_rl_repo", "/opt/pypackages"):
    if _p not in sys.path:
        sys.path.insert(0, _p)

import numpy as np

import concourse.bacc as bacc
import concourse.bass as bass
import concourse.tile as tile
from concourse import mybir
from concourse.bass_utils import run_bass_kernel_spmd

F32 = mybir.dt.float32
BF16 = mybir.dt.bfloat16
ALU = mybir.AluOpType
ACTF = mybir.ActivationFunctionType

B, T, H, K = 32, 2048, 1024, 11
PAD = (K - 1) // 2
NCORES = 8
BPC = B // NCORES      # batches per core
P = 128                # SBUF partitions
NCH = T // P           # 16 t-chunks of 128
G = 4                  # chunks per DMA
NG = NCH // G
H0 = 512               # psum-bank half of H

_CACHE = {}


def _build_nc():
    nc = bacc.Bacc(None, target_bir_lowering=False)

    eh_d = nc.declare_dram_parameter("eh", [BPC, T, H], F32, isOutput=False)
    dhx_d = nc.declare_dram_parameter("dhx", [BPC, H], F32, isOutput=False)
    axp_d = nc.declare_dram_parameter("axp", [BPC, P, NCH + 2], F32, isOutput=False)
    band_d = nc.declare_dram_parameter("band", [P, 3 * P], F32, isOutput=False)
    ident_d = nc.declare_dram_parameter("ident", [P, P], F32, isOutput=False)
    sx_d = nc.declare_dram_parameter("sx", [BPC, H], F32, isOutput=True)
    axo_d = nc.declare_dram_parameter("axn", [BPC, T], F32, isOutput=True)

    with tile.TileContext(nc) as tc:
        with (
            tc.tile_pool(name="pers", bufs=1) as pers,
            tc.tile_pool(name="ehp", bufs=4) as ehp,
            tc.tile_pool(name="scrp", bufs=2) as scrp,
            tc.tile_pool(name="bfp", bufs=2 * NCH) as bfp,
            tc.tile_pool(name="smallp", bufs=2) as smallp,
            tc.tile_pool(name="outp", bufs=2) as outp,
            tc.tile_pool(name="props", bufs=2, space="PSUM") as props,
            tc.tile_pool(name="sxps", bufs=2, space="PSUM") as sxps,
            tc.tile_pool(name="smps", bufs=3, space="PSUM") as smps,
        ):
            ident = pers.tile([P, P], F32, name="ident", tag="ident")
            band = pers.tile([P, 3 * P], F32, name="band", tag="band")
            ones_row = pers.tile([1, P], F32, name="ones_row", tag="ones_row")
            ones_col = pers.tile([P, 1], F32, name="ones_col", tag="ones_col")
            nc.sync.dma_start(out=ident[:], in_=ident_d[:])
            nc.sync.dma_start(out=band[:], in_=band_d[:])
            nc.vector.memset(ones_row[:], 1.0)
            nc.vector.memset(ones_col[:], 1.0)

            dhxrow, axp_t, dhxbc, rdhx, loc_sb, pax, pxt, ex, axn = (
                {} for _ in range(9)
            )
            for b in range(BPC):
                dhxrow[b] = scrp.tile([1, H], F32, name=f"dhxrow{b}", tag="scr")
                axp_t[b] = pers.tile([P, NCH + 2], F32, name=f"axp{b}", tag=f"axp{b}")
                dhxbc[b] = pers.tile([P, H], F32, name=f"dhxbc{b}", tag=f"dhxbc{b}")
                rdhx[b] = pers.tile([1, H], F32, name=f"rdhx{b}", tag=f"rdhx{b}")
                loc_sb[b] = pers.tile([P, NCH], F32, name=f"loc{b}", tag=f"loc{b}")
                pax[b] = pers.tile([P, NCH], F32, name=f"pax{b}", tag=f"pax{b}")
                pxt[b] = pers.tile([P, NCH], F32, name=f"pxt{b}", tag=f"pxt{b}")
                ex[b] = pers.tile([P, NCH], F32, name=f"ex{b}", tag=f"ex{b}")
                axn[b] = pers.tile([P, NCH], F32, name=f"axn{b}", tag=f"axn{b}")
                nc.sync.dma_start(out=dhxrow[b][:], in_=dhx_d[b : b + 1, :])
                nc.sync.dma_start(out=axp_t[b][:], in_=axp_d[b])

            # prologue: dhx broadcast to all partitions + conv location score
            for b in range(BPC):
                for hh in range(2):
                    bc_ps = props.tile([P, H0], F32, name="pro", tag="pro")
                    nc.tensor.matmul(
                        bc_ps[:],
                        lhsT=ones_row[:],
                        rhs=dhxrow[b][:, hh * H0 : (hh + 1) * H0],
                        start=True,
                        stop=True,
                    )
                    nc.scalar.copy(dhxbc[b][:, hh * H0 : (hh + 1) * H0], bc_ps[:])
                loc_ps = props.tile([P, NCH], F32, name="pro", tag="pro")
                for m in range(3):
                    nc.tensor.matmul(
                        loc_ps[:],
                        lhsT=band[:, m * P : (m + 1) * P],
                        rhs=axp_t[b][:, m : m + NCH],
                        start=(m == 0),
                        stop=(m == 2),
                    )
                nc.scalar.copy(loc_sb[b][:], loc_ps[:])
                nc.vector.reciprocal(rdhx[b][:], dhxrow[b][:])

            for b in range(BPC):
                ehv = eh_d[b].rearrange("(c p) h -> p c h", p=P)
                bigs = []
                for g in range(NG):
                    tl = ehp.tile([P, G, H], F32, name="eh", tag="eh")
                    nc.sync.dma_start(out=tl[:], in_=ehv[:, g * G : (g + 1) * G, :])
                    bigs.append(tl)

                # content score: prod = eh * dhx (DVE, f32); ACT reduces the f32
                # product into pax while emitting the bf16 product for the
                # weighted sum
                pbfs = []
                for c in range(NCH):
                    g, cl = divmod(c, G)
                    scr = scrp.tile([P, H], F32, name="scr", tag="scr")
                    nc.vector.tensor_tensor(
                        out=scr[:], in0=bigs[g][:, cl, :], in1=dhxbc[b][:], op=ALU.mult
                    )
                    pbf = bfp.tile([P, H], BF16, name="pbf", tag="pbf")
                    nc.scalar.activation(
                        pbf[:], scr[:], ACTF.Copy, accum_out=pax[b][:, c : c + 1]
                    )
                    pbfs.append(pbf)
                nc.vector.tensor_add(pxt[b][:], pax[b][:], loc_sb[b][:])

                # softmax over all 2048 logits (partition dim folded via PE)
                rowmax = smallp.tile([P, 1], F32, name="rowmax", tag="rowmax")
                nc.vector.tensor_reduce(
                    rowmax[:], pxt[b][:], axis=mybir.AxisListType.X, op=ALU.max
                )
                rm_ps = smps.tile([1, P], F32, name="sm", tag="sm")
                nc.tensor.transpose(rm_ps[:], rowmax[:], ident[:])
                ngmax = smallp.tile([1, 1], F32, name="ngmax", tag="ngmax")
                nc.vector.tensor_reduce(
                    ngmax[:],
                    rm_ps[:],
                    axis=mybir.AxisListType.X,
                    op=ALU.max,
                    negate=True,
                )
                nm_ps = smps.tile([P, 1], F32, name="sm", tag="sm")
                nc.tensor.matmul(
                    nm_ps[:], lhsT=ones_row[:], rhs=ngmax[:], start=True, stop=True
                )
                nmax = smallp.tile([P, 1], F32, name="nmax", tag="nmax")
                nc.scalar.copy(nmax[:], nm_ps[:])

                sumrow = smallp.tile([P, 1], F32, name="sumrow", tag="sumrow")
                nc.scalar.activation(
                    ex[b][:], pxt[b][:], ACTF.Exp, bias=nmax[:], accum_out=sumrow[:]
                )
                st_ps = smps.tile([1, 1], F32, name="sm", tag="sm")
                nc.tensor.matmul(
                    st_ps[:], lhsT=ones_col[:], rhs=sumrow[:], start=True, stop=True
                )
                rtot = smallp.tile([1, 1], F32, name="rtot", tag="rtot")
                nc.vector.reciprocal(rtot[:], st_ps[:])
                rb_ps = smps.tile([P, 1], F32, name="sm", tag="sm")
                nc.tensor.matmul(
                    rb_ps[:], lhsT=ones_row[:], rhs=rtot[:], start=True, stop=True
                )
                rbc = smallp.tile([P, 1], F32, name="rbc", tag="rbc")
                nc.scalar.copy(rbc[:], rb_ps[:])
                nc.scalar.activation(axn[b][:], ex[b][:], ACTF.Copy, scale=rbc[:])
                axn_bf = smallp.tile([P, NCH], BF16, name="axn_bf", tag="axn_bf")
                nc.scalar.copy(axn_bf[:], axn[b][:])

                # weighted sum over the bf16 product (bf16, full PE rate);
                # the trailing divide by dhx undoes the product's dhx factor
                sx_ps = [sxps.tile([1, H0], F32, name="sx", tag="sx") for _ in range(2)]
                for c in range(NCH):
                    for hh in range(2):
                        nc.tensor.matmul(
                            sx_ps[hh][:],
                            lhsT=axn_bf[:, c : c + 1],
                            rhs=pbfs[c][:, hh * H0 : (hh + 1) * H0],
                            start=(c == 0),
                            stop=(c == NCH - 1),
                        )
                sxr = outp.tile([1, H], F32, name="sxr", tag="sxr")
                for hh in range(2):
                    nc.scalar.copy(sxr[:, hh * H0 : (hh + 1) * H0], sx_ps[hh][:])
                sxs = outp.tile([1, H], F32, name="sxs", tag="sxs")
                nc.vector.tensor_mul(sxs[:], sxr[:], rdhx[b][:])
                nc.sync.dma_start(out=sx_d[b : b + 1, :], in_=sxs[:])

                # ax_new back to t-major rows for a contiguous store
                axt_ps = smps.tile([NCH, P], F32, name="sm", tag="sm")
                nc.tensor.transpose(axt_ps[:], axn[b][:], ident[:])
                axr = outp.tile([NCH, P], F32, name="axr", tag="axr")
                nc.scalar.copy(axr[:], axt_ps[:])
                nc.sync.dma_start(
                    out=axo_d[b].rearrange("(c p) -> c p", p=P), in_=axr[:]
                )

    nc.compile()
    return nc


def _get_nc():
    if "nc" not in _CACHE:
        _CACHE["nc"] = _build_nc()
    return _CACHE["nc"]


def _host_prep(eh, dhx, ax, conv_w):
    eh = np.ascontiguousarray(eh, dtype=np.float32)
    dhx = np.ascontiguousarray(dhx, dtype=np.float32).reshape(B, H)
    ax = np.ascontiguousarray(ax, dtype=np.float32)
    w = np.asarray(conv_w, dtype=np.float32).reshape(K)

    # conv as matmul: banded shift matrices (pure placement of the 11 taps)
    band = np.zeros((P, 3 * P), np.float32)
    qq = np.arange(P)[:, None]
    pp = np.arange(P)[None, :]
    for m, o in enumerate((-1, 0, 1)):
        j = qq + P * o - pp + PAD
        mask = (j >= 0) & (j < K)
        band[:, m * P : (m + 1) * P] = np.where(mask, w[np.clip(j, 0, K - 1)], 0.0)

    # ax transposed to (partition, chunk) with a zero chunk on each side
    axp = np.zeros((B, P, NCH + 2), np.float32)
    axp[:, :, 1 : NCH + 1] = ax.reshape(B, NCH, P).transpose(0, 2, 1)

    ident = np.eye(P, dtype=np.float32)
    return eh, dhx, axp, band, ident


def _run(eh, dhx, ax, conv_w, **spmd_kwargs):
    eh, dhx, axp, band, ident = _host_prep(eh, dhx, ax, conv_w)
    nc = _get_nc()
    in_maps = []
    for i in range(NCORES):
        sl = slice(i * BPC, (i + 1) * BPC)
        in_maps.append(
            {
                "eh": eh[sl],
                "dhx": dhx[sl],
                "axp": axp[sl],
                "band": band,
                "ident": ident,
            }
        )
    out = run_bass_kernel_spmd(nc, in_maps, core_ids=list(range(NCORES)), **spmd_kwargs)
    res = out.results
    sx = np.concatenate([np.asarray(r["sx"]) for r in res], axis=0)
    axn = np.concatenate([np.asarray(r["axn"]) for r in res], axis=0)
    sx = sx.reshape(B, 1, H).astype(np.float32)
    axn = axn.reshape(B, T).astype(np.float32)
    return (sx, axn), out


def kernel(eh, dhx, ax, conv_w, conv_b):
    # conv_b shifts every logit equally -> cancels in softmax; unused on device
    (sx, axn), _ = _run(eh, dhx, ax, conv_w)
    return (sx, axn)


# revision 19
# speedup vs baseline: 1.2505x; 1.2505x over previous
"""Trainium2 Bass kernel for location-sensitive attention (content score +
1-channel conv location score + softmax + weighted sum), data-parallel over
the batch dim across 8 NeuronCores.

Per core (4 batches): for each batch b
  prod[t,h] = eh[t,h] * dhx[h]                  (DVE mult, f32 -> scratch)
  pax[t]    = sum_h prod[t,h] + conv(ax)[t]     (ACT Copy w/ fp32 accum_out;
                                                 the Copy's main output is the
                                                 SAME product cast to bf16)
  ax_new    = softmax(pax)                      (ACT exp+rowsum fused, PE for
                                                 cross-partition folds)
  sx[h]     = (sum_t ax_new[t] * prod_bf16[t,h]) / dhx[h]
                                                (PE bf16 matmul over the bf16
                                                 product; one divide at the end
                                                 undoes the dhx factor)

eh is streamed from HBM exactly once (8 MiB/batch); only the bf16 product stays
resident for the weighted sum. DMA of batch b+1 overlaps compute of batch b.
"""

import sys

for _p in ("/opt/trn_rl_repo", "/opt/pypackages"):
    if _p not in sys.path:
        sys.path.insert(0, _p)

import numpy as np

import concourse.bacc as bacc
import concourse.bass as bass
import concourse.tile as tile
from concourse import mybir
from concourse.bass_utils import run_bass_kernel_spmd

F32 = mybir.dt.float32
BF16 = mybir.dt.bfloat16
ALU = mybir.AluOpType
ACTF = mybir.ActivationFunctionType

B, T, H, K = 32, 2048, 1024, 11
PAD = (K - 1) // 2
NCORES = 8
BPC = B // NCORES      # batches per core
P = 128                # SBUF partitions
NCH = T // P           # 16 t-chunks of 128
G = 4                  # chunks per DMA
NG = NCH // G
H0 = 512               # psum-bank half of H

_CACHE = {}


def _build_nc():
    nc = bacc.Bacc(None, target_bir_lowering=False)

    eh_d = nc.declare_dram_parameter("eh", [BPC, T, H], F32, isOutput=False)
    dhx_d = nc.declare_dram_parameter("dhx", [BPC, H], F32, isOutput=False)
    axp_d = nc.declare_dram_parameter("axp", [BPC, P, NCH + 2], F32, isOutput=False)
    band_d = nc.declare_dram_parameter("band", [P, 3 * P], F32, isOutput=False)
    ident_d = nc.declare_dram_parameter("ident", [P, P], F32, isOutput=False)
    sx_d = nc.declare_dram_parameter("sx", [BPC, H], F32, isOutput=True)
    axo_d = nc.declare_dram_parameter("axn", [BPC, T], F32, isOutput=True)

    with tile.TileContext(nc) as tc:
        with (
            tc.tile_pool(name="pers", bufs=1) as pers,
            tc.tile_pool(name="ehp", bufs=4) as ehp,
            tc.tile_pool(name="scrp", bufs=2) as scrp,
            tc.tile_pool(name="bfp", bufs=2 * NCH) as bfp,
            tc.tile_pool(name="smallp", bufs=2) as smallp,
            tc.tile_pool(name="outp", bufs=2) as outp,
            tc.tile_pool(name="props", bufs=2, space="PSUM") as props,
            tc.tile_pool(name="sxps", bufs=2, space="PSUM") as sxps,
            tc.tile_pool(name="smps", bufs=3, space="PSUM") as smps,
        ):
            ident = pers.tile([P, P], F32, name="ident", tag="ident")
            band = pers.tile([P, 3 * P], F32, name="band", tag="band")
            ones_row = pers.tile([1, P], F32, name="ones_row", tag="ones_row")
            ones_col = pers.tile([P, 1], F32, name="ones_col", tag="ones_col")
            nc.sync.dma_start(out=ident[:], in_=ident_d[:])
            nc.sync.dma_start(out=band[:], in_=band_d[:])
            nc.vector.memset(ones_row[:], 1.0)
            nc.vector.memset(ones_col[:], 1.0)

            dhxrow, axp_t, dhxbc, rdhx, loc_sb, pax, pxt, ex, axn = (
                {} for _ in range(9)
            )
            for b in range(BPC):
                dhxrow[b] = scrp.tile([1, H], F32, name=f"dhxrow{b}", tag="scr")
                axp_t[b] = pers.tile([P, NCH + 2], F32, name=f"axp{b}", tag=f"axp{b}")
                dhxbc[b] = pers.tile([P, H], F32, name=f"dhxbc{b}", tag=f"dhxbc{b}")
                rdhx[b] = pers.tile([1, H], F32, name=f"rdhx{b}", tag=f"rdhx{b}")
                loc_sb[b] = pers.tile([P, NCH], F32, name=f"loc{b}", tag=f"loc{b}")
                pax[b] = pers.tile([P, NCH], F32, name=f"pax{b}", tag=f"pax{b}")
                pxt[b] = pers.tile([P, NCH], F32, name=f"pxt{b}", tag=f"pxt{b}")
                ex[b] = pers.tile([P, NCH], F32, name=f"ex{b}", tag=f"ex{b}")
                axn[b] = pers.tile([P, NCH], F32, name=f"axn{b}", tag=f"axn{b}")
                nc.sync.dma_start(out=dhxrow[b][:], in_=dhx_d[b : b + 1, :])
                nc.sync.dma_start(out=axp_t[b][:], in_=axp_d[b])

            # prologue: dhx broadcast to all partitions + conv location score
            for b in range(BPC):
                for hh in range(2):
                    bc_ps = props.tile([P, H0], F32, name="pro", tag="pro")
                    nc.tensor.matmul(
                        bc_ps[:],
                        lhsT=ones_row[:],
                        rhs=dhxrow[b][:, hh * H0 : (hh + 1) * H0],
                        start=True,
                        stop=True,
                    )
                    nc.scalar.copy(dhxbc[b][:, hh * H0 : (hh + 1) * H0], bc_ps[:])
                loc_ps = props.tile([P, NCH], F32, name="pro", tag="pro")
                for m in range(3):
                    nc.tensor.matmul(
                        loc_ps[:],
                        lhsT=band[:, m * P : (m + 1) * P],
                        rhs=axp_t[b][:, m : m + NCH],
                        start=(m == 0),
                        stop=(m == 2),
                    )
                nc.scalar.copy(loc_sb[b][:], loc_ps[:])
                # 1/dhx on 128 lanes (a (1,H) reciprocal would crawl on one
                # lane), folded back to a row via PE transposes
                dhx8 = smallp.tile([P, H // P], F32, name="dhx8", tag="dhx8")
                nc.sync.dma_start(
                    out=dhx8[:], in_=dhx_d[b].rearrange("(k p) -> p k", p=P)
                )
                rdhx8 = smallp.tile([P, H // P], F32, name="rdhx8", tag="rdhx8")
                nc.vector.reciprocal(rdhx8[:], dhx8[:])
                for hh in range(2):
                    rps = props.tile([1, H0], F32, name="pro", tag="pro")
                    for j in range(4):
                        nc.tensor.transpose(
                            rps[:, j * P : (j + 1) * P],
                            rdhx8[:, hh * 4 + j : hh * 4 + j + 1],
                            ident[:],
                        )
                    nc.scalar.copy(rdhx[b][:, hh * H0 : (hh + 1) * H0], rps[:])

            pbfs_all = {}
            axn_bf_all = {}

            def s1(b):
                # stream eh, multiply by dhx (DVE), reduce the f32 product into
                # pax on ACT while emitting the bf16 product for the wsum
                ehv = eh_d[b].rearrange("(c p) h -> p c h", p=P)
                bigs = []
                for g in range(NG):
                    tl = ehp.tile([P, G, H], F32, name="eh", tag="eh")
                    nc.sync.dma_start(out=tl[:], in_=ehv[:, g * G : (g + 1) * G, :])
                    bigs.append(tl)
                pbfs = []
                for c in range(NCH):
                    g, cl = divmod(c, G)
                    scr = scrp.tile([P, H], F32, name="scr", tag="scr")
                    nc.vector.tensor_tensor(
                        out=scr[:], in0=bigs[g][:, cl, :], in1=dhxbc[b][:], op=ALU.mult
                    )
                    pbf = bfp.tile([P, H], BF16, name="pbf", tag="pbf")
                    nc.scalar.activation(
                        pbf[:], scr[:], ACTF.Copy, accum_out=pax[b][:, c : c + 1]
                    )
                    pbfs.append(pbf)
                pbfs_all[b] = pbfs

            def s2(b):
                # softmax over all 2048 logits (partition dim folded via PE)
                nc.vector.tensor_add(pxt[b][:], pax[b][:], loc_sb[b][:])
                rowmax = smallp.tile([P, 1], F32, name="rowmax", tag="rowmax")
                nc.vector.tensor_reduce(
                    rowmax[:], pxt[b][:], axis=mybir.AxisListType.X, op=ALU.max
                )
                rm_ps = smps.tile([1, P], F32, name="sm", tag="sm")
                nc.tensor.transpose(rm_ps[:], rowmax[:], ident[:])
                ngmax = smallp.tile([1, 1], F32, name="ngmax", tag="ngmax")
                nc.vector.tensor_reduce(
                    ngmax[:],
                    rm_ps[:],
                    axis=mybir.AxisListType.X,
                    op=ALU.max,
                    negate=True,
                )
                nm_ps = smps.tile([P, 1], F32, name="sm", tag="sm")
                nc.tensor.matmul(
                    nm_ps[:], lhsT=ones_row[:], rhs=ngmax[:], start=True, stop=True
                )
                nmax = smallp.tile([P, 1], F32, name="nmax", tag="nmax")
                nc.scalar.copy(nmax[:], nm_ps[:])
                sumrow = smallp.tile([P, 1], F32, name="sumrow", tag="sumrow")
                nc.scalar.activation(
                    ex[b][:], pxt[b][:], ACTF.Exp, bias=nmax[:], accum_out=sumrow[:]
                )
                st_ps = smps.tile([1, 1], F32, name="sm", tag="sm")
                nc.tensor.matmul(
                    st_ps[:], lhsT=ones_col[:], rhs=sumrow[:], start=True, stop=True
                )
                rtot = smallp.tile([1, 1], F32, name="rtot", tag="rtot")
                nc.vector.reciprocal(rtot[:], st_ps[:])
                rb_ps = smps.tile([P, 1], F32, name="sm", tag="sm")
                nc.tensor.matmul(
                    rb_ps[:], lhsT=ones_row[:], rhs=rtot[:], start=True, stop=True
                )
                rbc = smallp.tile([P, 1], F32, name="rbc", tag="rbc")
                nc.scalar.copy(rbc[:], rb_ps[:])
                nc.scalar.activation(axn[b][:], ex[b][:], ACTF.Copy, scale=rbc[:])
                axn_bf = smallp.tile([P, NCH], BF16, name="axn_bf", tag="axn_bf")
                nc.scalar.copy(axn_bf[:], axn[b][:])
                axn_bf_all[b] = axn_bf

            def s3(b):
                # weighted sum over the bf16 product (bf16, full PE rate);
                # the trailing divide by dhx undoes the product's dhx factor
                pbfs = pbfs_all.pop(b)
                axn_bf = axn_bf_all.pop(b)
                sx_ps = [
                    sxps.tile([1, H0], F32, name="sx", tag="sx") for _ in range(2)
                ]
                for c in range(NCH):
                    for hh in range(2):
                        nc.tensor.matmul(
                            sx_ps[hh][:],
                            lhsT=axn_bf[:, c : c + 1],
                            rhs=pbfs[c][:, hh * H0 : (hh + 1) * H0],
                            start=(c == 0),
                            stop=(c == NCH - 1),
                        )
                sxr = outp.tile([1, H], F32, name="sxr", tag="sxr")
                for hh in range(2):
                    nc.scalar.copy(sxr[:, hh * H0 : (hh + 1) * H0], sx_ps[hh][:])
                sxs = outp.tile([1, H], F32, name="sxs", tag="sxs")
                nc.vector.tensor_mul(sxs[:], sxr[:], rdhx[b][:])
                nc.sync.dma_start(out=sx_d[b : b + 1, :], in_=sxs[:])
                axt_ps = smps.tile([NCH, P], F32, name="sm", tag="sm")
                nc.tensor.transpose(axt_ps[:], axn[b][:], ident[:])
                axr = outp.tile([NCH, P], F32, name="axr", tag="axr")
                nc.scalar.copy(axr[:], axt_ps[:])
                nc.sync.dma_start(
                    out=axo_d[b].rearrange("(c p) -> c p", p=P), in_=axr[:]
                )

            # software pipeline: batch b's wsum/outputs are emitted inside
            # batch b+1's multiply stream so their latency hides under it
            for b in range(BPC):
                s1(b)
                if b > 0:
                    s3(b - 1)
                s2(b)
            s3(BPC - 1)

    nc.compile()
    return nc


def _get_nc():
    if "nc" not in _CACHE:
        _CACHE["nc"] = _build_nc()
    return _CACHE["nc"]


def _host_prep(eh, dhx, ax, conv_w):
    eh = np.ascontiguousarray(eh, dtype=np.float32)
    dhx = np.ascontiguousarray(dhx, dtype=np.float32).reshape(B, H)
    ax = np.ascontiguousarray(ax, dtype=np.float32)
    w = np.asarray(conv_w, dtype=np.float32).reshape(K)

    # conv as matmul: banded shift matrices (pure placement of the 11 taps)
    band = np.zeros((P, 3 * P), np.float32)
    qq = np.arange(P)[:, None]
    pp = np.arange(P)[None, :]
    for m, o in enumerate((-1, 0, 1)):
        j = qq + P * o - pp + PAD
        mask = (j >= 0) & (j < K)
        band[:, m * P : (m + 1) * P] = np.where(mask, w[np.clip(j, 0, K - 1)], 0.0)

    # ax transposed to (partition, chunk) with a zero chunk on each side
    axp = np.zeros((B, P, NCH + 2), np.float32)
    axp[:, :, 1 : NCH + 1] = ax.reshape(B, NCH, P).transpose(0, 2, 1)

    ident = np.eye(P, dtype=np.float32)
    return eh, dhx, axp, band, ident


def _run(eh, dhx, ax, conv_w, **spmd_kwargs):
    eh, dhx, axp, band, ident = _host_prep(eh, dhx, ax, conv_w)
    nc = _get_nc()
    in_maps = []
    for i in range(NCORES):
        sl = slice(i * BPC, (i + 1) * BPC)
        in_maps.append(
            {
                "eh": eh[sl],
                "dhx": dhx[sl],
                "axp": axp[sl],
                "band": band,
                "ident": ident,
            }
        )
    out = run_bass_kernel_spmd(nc, in_maps, core_ids=list(range(NCORES)), **spmd_kwargs)
    res = out.results
    sx = np.concatenate([np.asarray(r["sx"]) for r in res], axis=0)
    axn = np.concatenate([np.asarray(r["axn"]) for r in res], axis=0)
    sx = sx.reshape(B, 1, H).astype(np.float32)
    axn = axn.reshape(B, T).astype(np.float32)
    return (sx, axn), out


def kernel(eh, dhx, ax, conv_w, conv_b):
    # conv_b shifts every logit equally -> cancels in softmax; unused on device
    (sx, axn), _ = _run(eh, dhx, ax, conv_w)
    return (sx, axn)
